# revision 2
# baseline (speedup 1.0000x reference)
"""Trainium2 Bass kernel for nn_GCNTopK2 (GCN + TopKPooling, 64 graphs x 1024
nodes), v2.

Graph-data-parallel over 8 NeuronCores (8 graphs/core). Aggregation runs as
dense per-graph adjacency-count matmuls in plain bf16 (counts exact in bf16;
rel-err budget 2e-2 makes hi/lo splits unnecessary). Everything stays in SBUF
(no DRAM spills); top-k per graph via fixed-count threshold bisection on
[8,1024] score tiles; BatchNorm stats via one small AllReduce per BN layer.
"""

import sys

import numpy as np

sys.path.insert(0, "/opt/trn_rl_repo")

import concourse.bacc as bacc  # noqa: E402
import concourse.tile as tile  # noqa: E402
from concourse import mybir  # noqa: E402
from concourse import bass2jax  # noqa: E402

import ml_dtypes  # noqa: E402

BF16 = ml_dtypes.bfloat16
F32 = mybir.dt.float32
BF = mybir.dt.bfloat16
U8 = mybir.dt.uint8

G = 64
NPG = 1024
DEG = 8
INF = 128
HID = 256
OUTF = 256
K1 = 512
K2 = 256
EPS = 1e-5
NCORES = 8
GPC = G // NCORES            # 8 graphs per core
NODES = GPC * NPG            # 8192 nodes per core
NCH = NODES // 512           # 16 chunks of 512 nodes
P = 128
BIG = 1.0e30
BISECT_ITERS = 16

AF = mybir.ActivationFunctionType
ALU = mybir.AluOpType
AX = mybir.AxisListType


# =========================================================================
# Device program
# =========================================================================
def _emit(ctx, tc, io, phase_limit=99):
    nc = tc.nc

    wp = ctx.enter_context(tc.tile_pool(name="wp", bufs=1))
    bigp = ctx.enter_context(tc.tile_pool(name="bigp", bufs=1))
    mstr = ctx.enter_context(tc.tile_pool(name="mstr", bufs=2))
    sml = ctx.enter_context(tc.tile_pool(name="sml", bufs=3))
    jkp = ctx.enter_context(tc.tile_pool(name="jkp", bufs=2))
    st = ctx.enter_context(tc.tile_pool(name="st", bufs=1))
    psA = ctx.enter_context(tc.tile_pool(name="psA", bufs=2, space="PSUM"))
    psD = ctx.enter_context(tc.tile_pool(name="psD", bufs=2, space="PSUM"))
    psZ = ctx.enter_context(tc.tile_pool(name="psZ", bufs=1, space="PSUM"))
    psM = ctx.enter_context(tc.tile_pool(name="psM", bufs=2, space="PSUM"))
    dpool = ctx.enter_context(tc.tile_pool(name="dpool", bufs=1, space="DRAM"))

    def dma(dst, src):
        nc.gpsimd.dma_start(out=dst, in_=src)

    # ---- weights / constants (SBUF resident) ----
    def ldw(name, shape, dt=BF):
        t = wp.tile(shape, dt, tag=name, name=name + "_sb")
        dma(t[:], io[name][:])
        return t

    wrel1 = ldw("wrel1", [P, HID])
    wroot1 = ldw("wroot1", [P, HID])
    wrel2 = ldw("wrel2", [P, 2, HID])
    wroot2 = ldw("wroot2", [P, 2, HID])
    wl = ldw("wl", [P, 4, OUTF])
    u1f = ldw("u1f", [P, 2], F32)
    u2g8 = ldw("u2g8", [P, 2, GPC, GPC])
    ones_row = ldw("ones_row", [1, P])
    ones_col8 = ldw("ones_col8", [P, GPC])
    ident = ldw("identity", [P, P])
    b1 = ldw("b1", [P, 2], F32)
    b2 = ldw("b2", [P, 2], F32)
    g1c = ldw("g1c", [P, 2], F32)
    bt1c = ldw("bt1c", [P, 2], F32)
    g2c = ldw("g2c", [P, 2], F32)
    bt2c = ldw("bt2c", [P, 2], F32)
    bl_rep = ldw("bl_rep", [GPC, OUTF], F32)

    # ---- big SBUF tiles (slot reuse via shared tags) ----
    x_nm = bigp.tile([P, GPC * 8, P], BF, tag="A", name="x_nm")
    xt = bigp.tile([P, NODES], BF, tag="B", name="xt")
    dma(x_nm[:], io["x_nm"][:])
    dma(xt[:], io["xt"][:])
    hT = [bigp.tile([P, NODES], BF, tag=t, name=f"hT{m}")
          for m, t in ((0, "C"), (1, "D"))]
    h1T = [bigp.tile([P, NODES], BF, tag=t, name=f"h1T{m}")
           for m, t in ((0, "E"), (1, "F"))]

    # ---- DRAM tiles for collectives ----
    cc1_i = dpool.tile([P, 4], F32, tag="cc1_i", name="cc1_i")
    cc1_o = dpool.tile([P, 4], F32, tag="cc1_o", name="cc1_o",
                       addr_space="Shared")
    cc2_i = dpool.tile([P, 4], F32, tag="cc2_i", name="cc2_i")
    cc2_o = dpool.tile([P, 4], F32, tag="cc2_o", name="cc2_o",
                       addr_space="Shared")
    svrow1_d = dpool.tile([1, NODES], BF, tag="svrow1_d", name="svrow1_d")
    mkrow1_d = dpool.tile([1, NODES], BF, tag="mkrow1_d", name="mkrow1_d")
    svrow2_d = dpool.tile([1, NODES], BF, tag="svrow2_d", name="svrow2_d")

    # accumulators
    s1acc = st.tile([P, 2, NCH], F32, tag="s1acc", name="s1acc")
    q1acc = st.tile([P, 2, NCH], F32, tag="q1acc", name="q1acc")
    r1max = st.tile([P, 2, NCH], F32, tag="r1max", name="r1max")
    r1sum = st.tile([P, 2, NCH], F32, tag="r1sum", name="r1sum")
    q2acc = st.tile([P, 2, NCH], F32, tag="q2acc", name="q2acc")
    r2max = st.tile([P, 2, NCH], F32, tag="r2max", name="r2max")
    r2sum = st.tile([P, 2, NCH], F32, tag="r2sum", name="r2sum")

    # ================= conv1 =================
    for g in range(GPC):
        for dh in range(2):
            nch = g * 2 + dh
            nsl = slice(nch * 512, (nch + 1) * 512)
            dsl = slice(dh * 512, (dh + 1) * 512)
            aggps = psA.tile([P, 512], F32, tag="agg", name="aggps")
            mt8 = mstr.tile([P, 8, 512], BF, tag="mt", name="mt")
            dma(mt8[:], io["m_adj"][g, dh])
            for sc in range(8):
                nc.tensor.matmul(aggps[:], x_nm[:, g * 8 + sc, :],
                                 mt8[:, sc, :],
                                 start=(sc == 0), stop=(sc == 7))
            agg_bf = sml.tile([P, 512], BF, tag="aggbf", name="agg_bf")
            nc.vector.tensor_copy(agg_bf[:], aggps[:])
            for mch in range(2):
                msl = slice(mch * P, (mch + 1) * P)
                hps = psD.tile([P, 512], F32, tag="hps", name="hps")
                nc.tensor.matmul(hps[:], wrel1[:, msl], agg_bf[:],
                                 start=True, stop=False)
                nc.tensor.matmul(hps[:], wroot1[:, msl], xt[:, nsl],
                                 start=False, stop=True)
                # h = gelu(hps + b1); also accumulate sum for BN1
                nc.scalar.activation(
                    hT[mch][:, nsl], hps[:], AF.Gelu,
                    bias=b1[:, mch:mch + 1],
                    accum_out=s1acc[:, mch, nch:nch + 1])
                jsq = jkp.tile([P, 512], BF, tag="jsq", name="jsq")
                nc.scalar.activation(jsq[:], hT[mch][:, nsl], AF.Square,
                                     accum_out=q1acc[:, mch, nch:nch + 1])

    if phase_limit <= 1:
        return

    # ================= BN helper =================
    def bn_affine(sacc, qacc, cc_i, cc_o, count, gg, bb, tg):
        ssum = st.tile([P, 2], F32, tag=tg + "ss", name=tg + "ss")
        qsum = st.tile([P, 2], F32, tag=tg + "qs", name=tg + "qs")
        nc.vector.tensor_reduce(ssum[:], sacc[:], axis=AX.X, op=ALU.add)
        nc.vector.tensor_reduce(qsum[:], qacc[:], axis=AX.X, op=ALU.add)
        stat4 = st.tile([P, 4], F32, tag=tg + "s4", name=tg + "s4")
        nc.vector.tensor_copy(stat4[:, 0:2], ssum[:])
        nc.vector.tensor_copy(stat4[:, 2:4], qsum[:])
        dma(cc_i[:], stat4[:])
        nc.gpsimd.collective_compute(
            "AllReduce", ALU.add, replica_groups=[list(range(NCORES))],
            ins=[cc_i[:]], outs=[cc_o[:]])
        st4r = st.tile([P, 4], F32, tag=tg + "s4r", name=tg + "s4r")
        dma(st4r[:], cc_o[:])
        m = st.tile([P, 2], F32, tag=tg + "m", name=tg + "m")
        nc.vector.tensor_scalar_mul(m[:], st4r[:, 0:2], 1.0 / count)
        var = st.tile([P, 2], F32, tag=tg + "var", name=tg + "var")
        nc.vector.tensor_scalar_mul(var[:], st4r[:, 2:4], 1.0 / count)
        mm = st.tile([P, 2], F32, tag=tg + "mm", name=tg + "mm")
        nc.vector.tensor_tensor(out=mm[:], in0=m[:], in1=m[:], op=ALU.mult)
        nc.vector.tensor_tensor(out=var[:], in0=var[:], in1=mm[:],
                                op=ALU.subtract)
        nc.vector.tensor_scalar_add(var[:], var[:], EPS)
        sq = st.tile([P, 2], F32, tag=tg + "sq", name=tg + "sq")
        nc.scalar.activation(sq[:], var[:], AF.Sqrt)
        r = st.tile([P, 2], F32, tag=tg + "r", name=tg + "r")
        nc.vector.reciprocal(r[:], sq[:])
        tmp = st.tile([P, 2], F32, tag=tg + "tmp", name=tg + "tmp")
        for _ in range(2):
            nc.vector.tensor_tensor(out=tmp[:], in0=r[:], in1=r[:],
                                    op=ALU.mult)
            nc.vector.tensor_tensor(out=tmp[:], in0=tmp[:], in1=var[:],
                                    op=ALU.mult)
            nc.vector.tensor_scalar(out=tmp[:], in0=tmp[:], scalar1=-0.5,
                                    scalar2=1.5, op0=ALU.mult, op1=ALU.add)
            nc.vector.tensor_tensor(out=r[:], in0=r[:], in1=tmp[:],
                                    op=ALU.mult)
        s = st.tile([P, 2], F32, tag=tg + "s", name=tg + "s")
        nc.vector.tensor_tensor(out=s[:], in0=gg[:], in1=r[:], op=ALU.mult)
        t = st.tile([P, 2], F32, tag=tg + "t", name=tg + "t")
        nc.vector.tensor_tensor(out=t[:], in0=m[:], in1=s[:], op=ALU.mult)
        nc.vector.tensor_tensor(out=t[:], in0=bb[:], in1=t[:], op=ALU.subtract)
        return s, t

    s1t, t1t = bn_affine(s1acc, q1acc, cc1_i, cc1_o, float(G * NPG),
                         g1c, bt1c, "b1_")

    if phase_limit <= 2:
        return

    # ================= z1 scores =================
    # z1 = u1.hbn = sum_f (u1*s1)_f h_f + sum_f u1_f t1_f
    wzf = st.tile([P, 2], F32, tag="wzf", name="wzf")
    nc.vector.tensor_tensor(out=wzf[:], in0=u1f[:], in1=s1t[:], op=ALU.mult)
    wz8 = st.tile([P, 2, GPC, GPC], BF, tag="wz8", name="wz8")
    nc.vector.memset(wz8[:], 0.0)
    for kc in range(2):
        for g in range(GPC):
            nc.vector.tensor_copy(wz8[:, kc, g, g:g + 1], wzf[:, kc:kc + 1])
    pb = st.tile([P, 2], BF, tag="pb", name="pb")
    ptmp = st.tile([P, 2], F32, tag="ptmp", name="ptmp")
    nc.vector.tensor_tensor(out=ptmp[:], in0=u1f[:], in1=t1t[:], op=ALU.mult)
    nc.vector.tensor_copy(pb[:], ptmp[:])
    c1ps = psM.tile([GPC, 2], F32, tag="svp", name="c1ps")
    nc.tensor.matmul(c1ps[:], ones_col8[:], pb[:], start=True, stop=True)
    c1v = st.tile([GPC, 1], F32, tag="c1v", name="c1v")
    nc.vector.tensor_reduce(c1v[:], c1ps[:], axis=AX.X, op=ALU.add)

    zps = psZ.tile([GPC, NPG], F32, tag="z", name="zps1")
    for half in range(2):
        hsl = slice(half * 512, (half + 1) * 512)
        for g in range(GPC):
            for kc in range(2):
                nc.tensor.matmul(
                    zps[:, hsl], wz8[:, kc, g, :],
                    hT[kc][:, g * NPG + half * 512: g * NPG + half * 512 + 512],
                    start=(g == 0 and kc == 0), stop=(g == GPC - 1 and kc == 1))
    zsb = st.tile([GPC, NPG], F32, tag="zsb", name="zsb")
    nc.vector.tensor_scalar(out=zsb[:], in0=zps[:], scalar1=c1v[:],
                            scalar2=None, op0=ALU.add)

    if phase_limit <= 3:
        return

    # ================= top-k threshold bisection =================
    def bisect(z, k, tg, lo_src, hi_src):
        lo = st.tile([GPC, 1], F32, tag="pk_lo", name=tg + "lo")
        hi = st.tile([GPC, 1], F32, tag="pk_hi", name=tg + "hi")
        t = st.tile([GPC, 1], F32, tag="pk_t", name=tg + "t")
        cnt = st.tile([GPC, 1], F32, tag="pk_cnt", name=tg + "cnt")
        cond = st.tile([GPC, 1], U8, tag="pk_cond", name=tg + "cond")
        ncnd = st.tile([GPC, 1], U8, tag="pk_ncnd", name=tg + "ncnd")
        nc.vector.tensor_reduce(lo[:], lo_src[:], axis=AX.X, op=ALU.min)
        nc.vector.tensor_scalar_add(lo[:], lo[:], -1.0)
        nc.vector.tensor_reduce(hi[:], hi_src[:], axis=AX.X, op=ALU.max)
        nc.vector.tensor_scalar_add(hi[:], hi[:], 1.0)
        for _ in range(BISECT_ITERS):
            nc.vector.tensor_scalar(out=t[:], in0=lo[:], scalar1=hi[:],
                                    scalar2=0.5, op0=ALU.add, op1=ALU.mult)
            jb = jkp.tile([GPC, NPG], BF, tag="jb", name=tg + "jb")
            nc.vector.tensor_scalar(out=jb[:], in0=z[:], scalar1=t[:],
                                    scalar2=0.0, op0=ALU.is_ge, op1=ALU.add,
                                    accum_out=cnt[:])
            nc.vector.tensor_scalar(out=cond[:], in0=cnt[:], scalar1=float(k),
                                    scalar2=None, op0=ALU.is_ge)
            nc.vector.tensor_scalar(out=ncnd[:], in0=cnt[:], scalar1=float(k),
                                    scalar2=None, op0=ALU.is_lt)
            nc.vector.copy_predicated(lo[:], cond[:], t[:])
            nc.vector.copy_predicated(hi[:], ncnd[:], t[:])
        return lo

    thr1 = bisect(zsb, K1, "p1_", zsb, zsb)
    mask1 = st.tile([GPC, NPG], BF, tag="mask1", name="mask1")
    nc.vector.tensor_scalar(out=mask1[:], in0=zsb[:], scalar1=thr1[:],
                            scalar2=None, op0=ALU.is_ge)
    mask1u = st.tile([GPC, NPG], U8, tag="mask1u", name="mask1u")
    nc.vector.tensor_scalar(out=mask1u[:], in0=zsb[:], scalar1=thr1[:],
                            scalar2=None, op0=ALU.is_ge)
    tb1 = st.tile([GPC, NPG], BF, tag="zx", name="tb1")
    nc.scalar.activation(tb1[:], zsb[:], AF.Tanh)
    sv1 = st.tile([GPC, NPG], BF, tag="sv", name="sv1")
    nc.vector.tensor_tensor(out=sv1[:], in0=tb1[:], in1=mask1[:], op=ALU.mult)
    # flatten per-graph rows to a DRAM row (matmul operands need base
    # partition 0/32/64, so they bounce through DRAM to partition 0)
    dma(svrow1_d[:], sv1[:])
    dma(mkrow1_d[:], mask1[:])

    if phase_limit <= 4:
        return

    # ================= pool1: h1 = hbn*sv, readouts, BN2 stats ============
    for nch in range(NCH):
        nsl = slice(nch * 512, (nch + 1) * 512)
        svc = sml.tile([1, 512], BF, tag="svc", name="svc1")
        dma(svc[:], svrow1_d[0:1, nsl])
        svps = psM.tile([P, 512], F32, tag="svp", name="svps")
        nc.tensor.matmul(svps[:], ones_row[:], svc[0:1, :],
                         start=True, stop=True)
        for mch in range(2):
            tmp = sml.tile([P, 512], F32, tag="bn1t", name="bn1t")
            nc.scalar.activation(tmp[:], hT[mch][:, nsl], AF.Identity,
                                 bias=t1t[:, mch:mch + 1],
                                 scale=s1t[:, mch:mch + 1])
            nc.vector.tensor_tensor(out=h1T[mch][:, nsl], in0=tmp[:],
                                    in1=svps[:], op=ALU.mult)
            nc.vector.tensor_reduce(r1sum[:, mch, nch:nch + 1],
                                    h1T[mch][:, nsl], axis=AX.X, op=ALU.add)
            jsq = jkp.tile([P, 512], BF, tag="jsq", name="jsq2")
            nc.scalar.activation(jsq[:], h1T[mch][:, nsl], AF.Square,
                                 accum_out=q2acc[:, mch, nch:nch + 1])
            nc.vector.tensor_reduce(r1max[:, mch, nch:nch + 1],
                                    h1T[mch][:, nsl], axis=AX.X, op=ALU.max)

    if phase_limit <= 5:
        return

    s2t, t2t = bn_affine(r1sum, q2acc, cc2_i, cc2_o, float(G * K1),
                         g2c, bt2c, "b2_")

    # ================= hh = gelu(bn2(h1)) * alive; transpose ==============
    hhT = [bigp.tile([P, NODES], BF, tag=t, name=f"hhT{m}")
           for m, t in ((0, "C"), (1, "D"))]
    for nch in range(NCH):
        nsl = slice(nch * 512, (nch + 1) * 512)
        mkc = sml.tile([1, 512], BF, tag="svc", name="mkc")
        dma(mkc[:], mkrow1_d[0:1, nsl])
        alps = psM.tile([P, 512], F32, tag="svp", name="alps")
        nc.tensor.matmul(alps[:], ones_row[:], mkc[0:1, :],
                         start=True, stop=True)
        for mch in range(2):
            ghc = sml.tile([P, 512], BF, tag="ghc", name="ghc")
            nc.scalar.activation(ghc[:], h1T[mch][:, nsl], AF.Gelu,
                                 bias=t2t[:, mch:mch + 1],
                                 scale=s2t[:, mch:mch + 1])
            nc.vector.tensor_tensor(out=hhT[mch][:, nsl], in0=ghc[:],
                                    in1=alps[:], op=ALU.mult)

    hh_nm = bigp.tile([P, GPC * 8, HID], BF, tag="A", name="hh_nm")
    for mch in range(2):
        for nb4 in range(NCH):
            tp = psM.tile([P, 512], BF, tag="svp", name="tp")
            for q in range(4):
                nc.tensor.transpose(
                    tp[:, q * P:(q + 1) * P],
                    hhT[mch][:, (nb4 * 4 + q) * P:(nb4 * 4 + q + 1) * P],
                    ident[:])
            nc.vector.tensor_copy(
                hh_nm[:, nb4 * 4:(nb4 + 1) * 4, mch * P:(mch + 1) * P],
                tp[:].rearrange("p (q c) -> p q c", q=4))

    if phase_limit <= 6:
        return

    # ================= conv2 + z2 =================
    h2T = [bigp.tile([P, NODES], BF, tag=t, name=f"h2T{m}")
           for m, t in ((0, "B"), (1, "H"))]
    for dh in range(2):
        dsl = slice(dh * 512, (dh + 1) * 512)
        for g in range(GPC):
            nch = g * 2 + dh
            nsl = slice(nch * 512, (nch + 1) * 512)
            a2ps = [psA.tile([P, 512], F32, tag="agg", name=f"a2ps{fc}")
                    for fc in range(2)]
            mt8 = mstr.tile([P, 8, 512], BF, tag="mt", name="mt2")
            dma(mt8[:], io["m_adj"][g, dh])
            for sc in range(8):
                for fc in range(2):
                    nc.tensor.matmul(
                        a2ps[fc][:], hh_nm[:, g * 8 + sc, fc * P:(fc + 1) * P],
                        mt8[:, sc, :], start=(sc == 0), stop=(sc == 7))
            a2sb = sml.tile([P, 2, 512], BF, tag="a2sb", name="a2sb")
            for fc in range(2):
                nc.vector.tensor_copy(a2sb[:, fc, :], a2ps[fc][:])
            for mch in range(2):
                msl = slice(mch * P, (mch + 1) * P)
                h2ps = psD.tile([P, 512], F32, tag="hps", name="h2ps")
                for kc in range(2):
                    nc.tensor.matmul(h2ps[:], wrel2[:, kc, msl],
                                     a2sb[:, kc, :],
                                     start=(kc == 0), stop=False)
                    nc.tensor.matmul(h2ps[:], wroot2[:, kc, msl],
                                     hhT[kc][:, nsl],
                                     start=False, stop=(kc == 1))
                nc.scalar.activation(h2T[mch][:, nsl], h2ps[:], AF.Identity,
                                     bias=b2[:, mch:mch + 1])

    if phase_limit <= 7:
        return

    # ================= pool2 =================
    zps2 = psZ.tile([GPC, NPG], F32, tag="z", name="zps2")
    for half in range(2):
        hsl = slice(half * 512, (half + 1) * 512)
        for g in range(GPC):
            for kc in range(2):
                nc.tensor.matmul(
                    zps2[:, hsl], u2g8[:, kc, g, :],
                    h2T[kc][:, g * NPG + half * 512: g * NPG + half * 512
                            + 512],
                    start=(g == 0 and kc == 0),
                    stop=(g == GPC - 1 and kc == 1))
    zsb2 = st.tile([GPC, NPG], F32, tag="zsb", name="zsb2")
    nc.vector.tensor_copy(zsb2[:], zps2[:])
    z2m = st.tile([GPC, NPG], F32, tag="z2m", name="z2m")
    nc.vector.memset(z2m[:], -BIG)
    nc.vector.copy_predicated(z2m[:], mask1u[:], zsb2[:])
    zpos = st.tile([GPC, NPG], F32, tag="zx", name="zpos")
    nc.vector.memset(zpos[:], BIG)
    nc.vector.copy_predicated(zpos[:], mask1u[:], zsb2[:])
    thr2 = bisect(z2m, K2, "p2_", zpos, z2m)
    mask2 = st.tile([GPC, NPG], BF, tag="mask1", name="mask2")
    nc.vector.tensor_scalar(out=mask2[:], in0=z2m[:], scalar1=thr2[:],
                            scalar2=None, op0=ALU.is_ge)
    tb2 = st.tile([GPC, NPG], BF, tag="sv", name="tb2")
    nc.scalar.activation(tb2[:], zsb2[:], AF.Tanh)
    sv2 = st.tile([GPC, NPG], BF, tag="sv2", name="sv2")
    nc.vector.tensor_tensor(out=sv2[:], in0=tb2[:], in1=mask2[:], op=ALU.mult)
    dma(svrow2_d[:], sv2[:])

    if phase_limit <= 8:
        return

    # ================= readout2 =================
    for nch in range(NCH):
        nsl = slice(nch * 512, (nch + 1) * 512)
        svc2 = sml.tile([1, 512], BF, tag="svc", name="svc2")
        dma(svc2[:], svrow2_d[0:1, nsl])
        svps2 = psM.tile([P, 512], F32, tag="svp", name="svps2")
        nc.tensor.matmul(svps2[:], ones_row[:], svc2[0:1, :],
                         start=True, stop=True)
        for mch in range(2):
            prod = jkp.tile([P, 512], F32, tag="prod", name="prod")
            nc.vector.tensor_tensor(out=prod[:], in0=h2T[mch][:, nsl],
                                    in1=svps2[:], op=ALU.mult)
            jsm = jkp.tile([P, 512], BF, tag="jsq", name="jsm")
            nc.scalar.activation(jsm[:], prod[:], AF.Identity,
                                 accum_out=r2sum[:, mch, nch:nch + 1])
            nc.vector.tensor_reduce(r2max[:, mch, nch:nch + 1], prod[:],
                                    axis=AX.X, op=ALU.max)

    # ================= final linear =================
    xc = st.tile([P, 4, GPC], F32, tag="xc", name="xc")
    tmpa = st.tile([P, GPC], F32, tag="tmpa", name="tmpa")
    tmpb = st.tile([P, GPC], F32, tag="tmpb", name="tmpb")
    for mch in range(2):
        nc.vector.tensor_reduce(
            tmpa[:], r1max[:, mch, :].rearrange("p (g d) -> p g d", d=2),
            axis=AX.X, op=ALU.max)
        nc.vector.tensor_reduce(
            tmpb[:], r2max[:, mch, :].rearrange("p (g d) -> p g d", d=2),
            axis=AX.X, op=ALU.max)
        nc.vector.tensor_tensor(out=xc[:, mch, :], in0=tmpa[:], in1=tmpb[:],
                                op=ALU.add)
        nc.vector.tensor_reduce(
            tmpa[:], r1sum[:, mch, :].rearrange("p (g d) -> p g d", d=2),
            axis=AX.X, op=ALU.add)
        nc.vector.tensor_scalar_mul(tmpa[:], tmpa[:], 1.0 / K1)
        nc.vector.tensor_reduce(
            tmpb[:], r2sum[:, mch, :].rearrange("p (g d) -> p g d", d=2),
            axis=AX.X, op=ALU.add)
        nc.vector.tensor_scalar_mul(tmpb[:], tmpb[:], 1.0 / K2)
        nc.vector.tensor_tensor(out=xc[:, 2 + mch, :], in0=tmpa[:],
                                in1=tmpb[:], op=ALU.add)
    xch = st.tile([P, 4, GPC], BF, tag="xch", name="xch")
    nc.vector.tensor_copy(xch[:], xc[:])
    ops_f = psD.tile([GPC, OUTF], F32, tag="hps", name="ops_f")
    for kc in range(4):
        nc.tensor.matmul(ops_f[:], xch[:, kc, :], wl[:, kc, :],
                         start=(kc == 0), stop=(kc == 3))
    out_sb = st.tile([GPC, OUTF], F32, tag="out_sb", name="out_sb")
    nc.vector.tensor_tensor(out=out_sb[:], in0=ops_f[:], in1=bl_rep[:],
                            op=ALU.add)
    dma(io["out"][:], out_sb[:])

    # small debug outputs
    dbg = st.tile([P, 8], F32, tag="dbg", name="dbg")
    nc.vector.tensor_copy(dbg[:, 0:2], s1t[:])
    nc.vector.tensor_copy(dbg[:, 2:4], t1t[:])
    nc.vector.tensor_copy(dbg[:, 4:6], s2t[:])
    nc.vector.tensor_copy(dbg[:, 6:8], t2t[:])
    dma(io["dbg"][:], dbg[:])
    thrs = st.tile([GPC, 2], F32, tag="thrs", name="thrs")
    nc.vector.tensor_copy(thrs[:, 0:1], thr1[:])
    nc.vector.tensor_copy(thrs[:, 1:2], thr2[:])
    dma(io["thrs"][:], thrs[:])


# =========================================================================
# Build
# =========================================================================
_CACHE = {}


def _build_program():
    import os
    phase_limit = int(os.environ.get("KPHASE", "99"))
    if "nc" in _CACHE:
        return _CACHE["nc"], _CACHE["io"]
    nc = bacc.Bacc("TRN2", target_bir_lowering=False, debug=False,
                   num_devices=NCORES)
    io = {}

    def din(name, shape, dt=BF):
        io[name] = nc.dram_tensor(name, shape, dt, kind="ExternalInput").ap()

    din("m_adj", [GPC, 2, P, 8, 512])
    din("x_nm", [P, GPC * 8, P])
    din("xt", [P, NODES])
    din("wrel1", [P, HID])
    din("wroot1", [P, HID])
    din("wrel2", [P, 2, HID])
    din("wroot2", [P, 2, HID])
    din("wl", [P, 4, OUTF])
    din("u1f", [P, 2], F32)
    din("u2g8", [P, 2, GPC, GPC])
    din("ones_row", [1, P])
    din("ones_col8", [P, GPC])
    din("identity", [P, P])
    din("b1", [P, 2], F32)
    din("b2", [P, 2], F32)
    din("g1c", [P, 2], F32)
    din("bt1c", [P, 2], F32)
    din("g2c", [P, 2], F32)
    din("bt2c", [P, 2], F32)
    din("bl_rep", [GPC, OUTF], F32)
    io["out"] = nc.dram_tensor("out", [GPC, OUTF], F32,
                               kind="ExternalOutput").ap()
    io["dbg"] = nc.dram_tensor("dbg", [P, 8], F32, kind="ExternalOutput").ap()
    io["thrs"] = nc.dram_tensor("thrs", [GPC, 2], F32,
                                kind="ExternalOutput").ap()

    from contextlib import ExitStack
    with tile.TileContext(nc) as tc:
        ctx = ExitStack()
        with ctx:
            _emit(ctx, tc, io, phase_limit)
    nc.compile()
    _CACHE["nc"] = nc
    _CACHE["io"] = io
    return nc, io


# =========================================================================
# Host-side input prep
# =========================================================================
def _chunk2(w):
    return np.ascontiguousarray(
        np.asarray(w, np.float32).reshape(2, 128, -1).transpose(1, 0, 2))


def _colplace(v):
    # v: [256] -> [128, 2, GPC, GPC] with chunk kc of v in column g (row g)
    vc = np.asarray(v, np.float32).reshape(2, 128).T  # [128, 2]
    out = np.zeros((128, 2, GPC, GPC), np.float32)
    for g in range(GPC):
        out[:, :, g, g] = vc
    return out.astype(BF16)


def make_in_maps(inputs):
    x = np.asarray(inputs["x"], np.float32)
    src = np.asarray(inputs["src"], np.int64)
    dst = np.asarray(inputs["dst"], np.int64)

    W_rel1 = np.asarray(inputs["W_rel1"], np.float32)
    b_rel1 = np.asarray(inputs["b_rel1"], np.float32)
    W_root1 = np.asarray(inputs["W_root1"], np.float32)
    g1 = np.asarray(inputs["g1"], np.float32)
    bt1 = np.asarray(inputs["bt1"], np.float32)
    p1 = np.asarray(inputs["p1"], np.float32)
    g2 = np.asarray(inputs["g2"], np.float32)
    bt2 = np.asarray(inputs["bt2"], np.float32)
    W_rel2 = np.asarray(inputs["W_rel2"], np.float32)
    b_rel2 = np.asarray(inputs["b_rel2"], np.float32)
    W_root2 = np.asarray(inputs["W_root2"], np.float32)
    p2 = np.asarray(inputs["p2"], np.float32)
    Wl = np.asarray(inputs["Wl"], np.float32)
    bl = np.asarray(inputs["bl"], np.float32)

    u1 = (p1 / np.float32(np.linalg.norm(p1))).astype(np.float32)
    u2 = (p2 / np.float32(np.linalg.norm(p2))).astype(np.float32)
    vrel2 = (W_rel2.astype(np.float64) @ u2.astype(np.float64)).astype(
        np.float32)
    vroot2 = (W_root2.astype(np.float64) @ u2.astype(np.float64)).astype(
        np.float32)
    c2 = float(u2.astype(np.float64) @ b_rel2.astype(np.float64))

    sh = {
        "wrel1": W_rel1.astype(BF16),
        "wroot1": W_root1.astype(BF16),
        "wrel2": _chunk2(W_rel2).astype(BF16),
        "wroot2": _chunk2(W_root2).astype(BF16),
        "wl": np.ascontiguousarray(
            Wl.reshape(4, 128, OUTF).transpose(1, 0, 2)).astype(BF16),
        "u1f": np.ascontiguousarray(u1.reshape(2, 128).T).astype(np.float32),
        "u2g8": _colplace(u2),
        "ones_row": np.ones((1, P), BF16),
        "ones_col8": np.ones((P, GPC), BF16),
        "identity": np.eye(P, dtype=BF16),
        "bl_rep": np.broadcast_to(bl, (GPC, OUTF)).astype(np.float32).copy(),
    }
    for nm, v in (("b1", b_rel1), ("b2", b_rel2), ("g1c", g1),
                  ("bt1c", bt1), ("g2c", g2), ("bt2c", bt2)):
        sh[nm] = np.ascontiguousarray(
            v.reshape(2, 128).T).astype(np.float32)

    assert np.all(src // NPG == dst // NPG), "edges must be graph-local"
    in_maps = []
    for c in range(NCORES):
        xs = x[c * NODES:(c + 1) * NODES]
        m = dict(sh)
        madj = np.zeros((GPC, NPG, NPG), np.float32)
        for gi in range(GPC):
            gg = c * GPC + gi
            e0, e1 = gg * NPG * DEG, (gg + 1) * NPG * DEG
            s_loc = src[e0:e1] - gg * NPG
            d_loc = dst[e0:e1] - gg * NPG
            cnts = np.bincount(s_loc * NPG + d_loc, minlength=NPG * NPG)
            assert cnts.max() <= 256
            madj[gi] = cnts.reshape(NPG, NPG)
        m["m_adj"] = np.ascontiguousarray(
            madj.reshape(GPC, 8, P, 2, 512).transpose(0, 3, 2, 1, 4)
        ).astype(BF16)
        xb = xs.astype(BF16)
        m["x_nm"] = np.ascontiguousarray(
            xb.reshape(GPC * 8, P, P).transpose(1, 0, 2))
        m["xt"] = np.ascontiguousarray(xb.T)
        in_maps.append(m)
    return in_maps


def kernel(**inputs):
    in_maps = make_in_maps(inputs)
    nc, io = _build_program()
    res = bass2jax.run_bass_via_pjrt(nc, in_maps, n_cores=NCORES)
    out = np.concatenate([res[c]["out"] for c in range(NCORES)], axis=0)
    return out.astype(np.float32)


if __name__ == "__main__":
    nc, io = _build_program()
    print("program built OK")
